# revision 43
# baseline (speedup 1.0000x reference)
"""Gated max/avg 2x2 pooling kernel for Trainium2 (8 NeuronCores, SPMD).

Reference computation (per 2x2 window over [B, H, W, C], stride 2):
    x1 = max(window), s = mean(window)
    xs = sum_ij mask[i, j] * window[i, j]   (per channel)
    z  = sigmoid(xs);  out = z * x1 + (1 - z) * s

Rewritten as  out = x1 - zb * d,  zb = sigmoid(-xs),  d = x1 - s, so the
linear algebra rides the (otherwise idle) PE:

  PE    pass 1: psum_xs = sum_k u_k*T_k  (diagonal stationaries u_k*I,
        T_k = window terms Ee,Eo,Oe,Oo); pass 2: psum_d = I@x1 +
        sum_k (-0.25*I)@T_k  = x1 - mean.
  ACT   zb = sigmoid(-f * psum_xs) straight out of PSUM (f = mask scale);
        d: PSUM -> SBUF fp16 copy.
  DVE   max pool (2 tensor_max, e-major intermediate) + final
        o = x1 - gq.
  GPSIMD  gq = zb * d.

A 3-stage software pipeline keeps every engine's in-order queue from
stalling on the cross-engine chain.  Inputs are staged fp16 on the host
(halves HBM read traffic); output returns fp16 and is upcast on host.
"""

import numpy as np

import concourse.bacc as bacc
import concourse.mybir as mybir
import concourse.tile as tile
from concourse.bass_utils import run_bass_kernel_spmd

F32 = mybir.dt.float32
F16 = mybir.dt.float16

B, H, W, C = 16, 256, 256, 64
N_CORES = 8
BPC = B // N_CORES          # batches per core
HO = H // 2                 # 128 output rows = SBUF partitions
NQ = 8                      # w-slices (tiles) per row
WQ = 16                     # output w per tile
N = WQ * C                  # 1024 free elems per partition per output tile
FD = 4 * N                  # input tile free dim (r2 * w16 * e2 * c64)
MMCH = 512                  # matmul moving-free chunk (PSUM bank)

LAST_EXEC_NS = None
LAST_RESULTS = None

_PROGRAM_CACHE = {}


def _build_program(bpc, nq, wq, ch):
    from contextlib import ExitStack

    n = wq * ch
    fd = 4 * n
    nch = n // MMCH             # psum chunks per tile

    nc = bacc.Bacc(
        "TRN2",
        target_bir_lowering=False,
        debug=False,
        enable_asserts=True,
        num_devices=N_CORES,
    )

    x = nc.dram_tensor("x", [bpc, HO, nq, fd], F16, kind="ExternalInput")
    wmat = nc.dram_tensor("wmat", [128, 6 * 128], F16, kind="ExternalInput")
    scal = nc.dram_tensor("scal", [128, 8], F32, kind="ExternalInput")
    out = nc.dram_tensor("out", [bpc, HO, nq, n], F16, kind="ExternalOutput")
    x_ap = x.ap()
    out_ap = out.ap()

    with tile.TileContext(nc) as tc, ExitStack() as ctx:
        pio = ctx.enter_context(tc.tile_pool(name="io", bufs=4))
        pbig = ctx.enter_context(tc.tile_pool(name="big", bufs=3))
        psm = ctx.enter_context(tc.tile_pool(name="small", bufs=5))
        pout = ctx.enter_context(tc.tile_pool(name="outp", bufs=3))
        pconst = ctx.enter_context(tc.tile_pool(name="const", bufs=1))
        ppsum = ctx.enter_context(tc.tile_pool(name="acc", bufs=2, space="PSUM"))

        Wt = pconst.tile([128, 6 * 128], F16)
        nc.sync.dma_start(Wt[:], wmat.ap()[:])
        Sc = pconst.tile([128, 8], F32)
        nc.sync.dma_start(Sc[:], scal.ap()[:])
        fneg_ap = Sc[:, 0:1]        # -f
        zero_ap = Sc[:, 1:2]
        Wd = [Wt[:, k * 128 : (k + 1) * 128] for k in range(6)]
        W_U = Wd[0:4]               # u_k * I
        W_Q = Wd[4]                 # -0.25 * I
        W_I = Wd[5]                 # I

        load_engines = [nc.sync, nc.scalar]
        load_rr = [0]

        def emit_load(b, q):
            EO = pio.tile([128, fd], F16, tag="EO")
            eng = load_engines[load_rr[0] % len(load_engines)]
            load_rr[0] += 1
            eng.dma_start(EO[:], x_ap[b, :, q, :])
            return dict(b=b, q=q, EO=EO)

        def terms_of(EO):
            EOv = EO[:].rearrange("p (r w e c) -> p r w e c", r=2, e=2, c=ch)

            def term(k, c0, nw):
                r, e = divmod(k, 2)
                return EOv[:, r, c0 : c0 + nw, e, :]

            return EOv, term

        wpc = MMCH // ch            # moving w's per chunk

        def emit_stage1(h):
            """xs psum + sigmoid; max pool."""
            b, q, EO = h["b"], h["q"], h["EO"]
            EOv, term = terms_of(EO)

            pxs = ppsum.tile([128, n], F32, tag="pxs")
            for cH in range(nch):
                for k in range(4):
                    nc.tensor.matmul(
                        pxs[:, cH * MMCH : (cH + 1) * MMCH],
                        W_U[k],
                        term(k, cH * wpc, wpc),
                        start=(k == 0),
                        stop=(k == 3),
                    )
            zb = psm.tile([128, n], F16, tag="zb")
            nc.scalar.activation(
                zb[:],
                pxs[:],
                mybir.ActivationFunctionType.Sigmoid,
                bias=zero_ap,
                scale=fneg_ap,
            )

            M1 = pbig.tile([128, 2 * n], F16, tag="M1")
            nc.vector.tensor_max(
                M1[:].rearrange("p (e w c) -> p w e c", e=2, c=ch),
                EOv[:, 0],
                EOv[:, 1],
            )
            x1 = psm.tile([128, n], F16, tag="x1")
            nc.vector.tensor_max(x1[:], M1[:, 0:n], M1[:, n : 2 * n])
            h.update(zb=zb, x1=x1)
            return h

        def emit_stage2(h):
            """d = x1 - mean: ACT seeds the psum with x1, PE accumulates
            the -0.25 window terms onto it."""
            EOv, term = terms_of(h["EO"])
            x1 = h["x1"]
            pd = ppsum.tile([128, n], F32, tag="pd")
            for cH in range(nch):
                nc.tensor.matmul(
                    pd[:, cH * MMCH : (cH + 1) * MMCH],
                    W_I,
                    x1[:, cH * MMCH : (cH + 1) * MMCH],
                    start=True,
                    stop=False,
                )
                for k in range(4):
                    nc.tensor.matmul(
                        pd[:, cH * MMCH : (cH + 1) * MMCH],
                        W_Q,
                        term(k, cH * wpc, wpc),
                        start=False,
                        stop=(k == 3),
                    )
            d = psm.tile([128, n], F16, tag="d")
            nc.scalar.copy(d[:], pd[:])
            h.update(d=d)
            return h

        def emit_stage3(h):
            """gq = zb*d (GPSIMD); o = x1 - gq; store."""
            gq = psm.tile([128, n], F16, tag="gq")
            nc.gpsimd.tensor_mul(gq[:], h["zb"][:], h["d"][:])
            o = pout.tile([128, n], F16, tag="o")
            nc.vector.tensor_sub(o[:], h["x1"][:], gq[:])
            nc.sync.dma_start(out_ap[h["b"], :, h["q"], :], o[:])

        tiles = [(b, q) for b in range(bpc) for q in range(nq)]
        ntiles = len(tiles)
        AHEAD = 3
        loaded = [emit_load(*tiles[i]) for i in range(min(AHEAD, ntiles))]
        s1q, s2q = [], []
        for i in range(ntiles):
            if i + AHEAD < ntiles:
                loaded.append(emit_load(*tiles[i + AHEAD]))
            s1q.append(emit_stage1(loaded.pop(0)))
            if len(s1q) >= 3:
                s2q.append(emit_stage2(s1q.pop(0)))
            if len(s2q) >= 2:
                emit_stage3(s2q.pop(0))
        while s1q:
            s2q.append(emit_stage2(s1q.pop(0)))
        while s2q:
            emit_stage3(s2q.pop(0))

    nc.compile()
    return nc


def _get_program(key):
    if key not in _PROGRAM_CACHE:
        _PROGRAM_CACHE[key] = _build_program(*key)
    return _PROGRAM_CACHE[key]


def _mask_consts(mask):
    """wmat [128, 6*128] f16 diagonal stationaries (u0..u3, -0.25*I, I)
    and scal [128, 8] f32 (-f, 0)."""
    m = np.asarray(mask, np.float64).reshape(-1)  # m00 m01 m10 m11 = Ee Eo Oe Oo
    f = float(m[np.argmax(np.abs(m))])
    if f == 0.0:
        f = 1.0
    u = m / f
    wmat = np.zeros((128, 6 * 128), np.float16)
    idx = np.arange(128)
    for k in range(4):
        wmat[idx, k * 128 + idx] = np.float16(u[k])
    wmat[idx, 4 * 128 + idx] = np.float16(-0.25)
    wmat[idx, 5 * 128 + idx] = np.float16(1.0)
    scal = np.zeros((128, 8), np.float32)
    scal[:, 0] = -f
    return wmat, scal


def kernel(x, mask):
    import os

    global LAST_EXEC_NS, LAST_RESULTS

    x = np.asarray(x)
    mask = np.asarray(mask)
    assert x.shape == (B, H, W, C), x.shape
    in_dtype = x.dtype

    wmat, scal = _mask_consts(mask)
    nc = _get_program((BPC, NQ, WQ, C))

    # stage as [b, h, q, r, w, e, c] -> fp16
    xs = np.asarray(x, np.float32).reshape(B, HO, 2, NQ, WQ, 2, C)
    xt = xs.transpose(0, 1, 3, 2, 4, 5, 6)
    xv = np.ascontiguousarray(xt).astype(np.float16).reshape(B, HO, NQ, FD)

    in_maps = [
        {"x": xv[i * BPC : (i + 1) * BPC], "wmat": wmat, "scal": scal}
        for i in range(N_CORES)
    ]

    trace = os.environ.get("KERNEL_TRACE", "0") == "1"
    res = run_bass_kernel_spmd(
        nc, in_maps, core_ids=list(range(N_CORES)), trace=trace
    )
    LAST_EXEC_NS = res.exec_time_ns
    LAST_RESULTS = res

    parts = [
        r["out"].reshape(BPC, HO, NQ * WQ, C).astype(np.float32)
        for r in res.results
    ]
    full = np.concatenate(parts, axis=0)
    return full.astype(in_dtype, copy=False)


def _numpy_reference(x, mask):
    xr = x.reshape(x.shape[0], x.shape[1] // 2, 2, x.shape[2] // 2, 2, x.shape[3])
    x1 = xr.max(axis=(2, 4))
    x2 = xr.mean(axis=(2, 4))
    xs = np.einsum("bhiwjc,ij->bhwc", xr, mask)
    z = 1.0 / (1.0 + np.exp(-xs))
    return z * x1 + (1.0 - z) * x2


if __name__ == "__main__":
    # Small-scale CoreSim self-test (no hardware needed).
    from concourse.bass_interp import CoreSim

    rng = np.random.default_rng(0)
    bpc_s, nq_s = 1, 2
    w_s = nq_s * WQ * 2
    xs_np = rng.standard_normal((bpc_s, H, w_s, C)).astype(np.float32)
    mask_np = (rng.standard_normal((2, 2)) * 0.5).astype(np.float32)

    wmat_s, scal_s = _mask_consts(mask_np)
    nc = _build_program(bpc_s, nq_s, WQ, C)
    sim = CoreSim(nc, trace=False)
    xr = xs_np.reshape(bpc_s, HO, 2, nq_s, WQ, 2, C).transpose(0, 1, 3, 2, 4, 5, 6)
    sim.tensor("x")[:] = (
        np.ascontiguousarray(xr).astype(np.float16).reshape(bpc_s, HO, nq_s, FD)
    )
    sim.tensor("wmat")[:] = wmat_s
    sim.tensor("scal")[:] = scal_s
    sim.simulate()
    got = sim.tensor("out").reshape(bpc_s, HO, nq_s * WQ, C).astype(np.float64)
    want = _numpy_reference(xs_np.astype(np.float64), mask_np.astype(np.float64))
    err = np.abs(got - want)
    rel = err.max() / np.abs(want).max()
    print("CoreSim selftest: max abs err", err.max(), "rel", rel)
    assert rel < 5e-3, rel
    print("PASS")


# revision 45
# speedup vs baseline: 1.1056x; 1.1056x over previous
"""Gated max/avg 2x2 pooling kernel for Trainium2 (8 NeuronCores, SPMD).

Reference computation (per 2x2 window over [B, H, W, C], stride 2):
    x1 = max(window), s = mean(window)
    xs = sum_ij mask[i, j] * window[i, j]   (per channel)
    z  = sigmoid(xs);  out = z * x1 + (1 - z) * s

Rewritten as  out = x1 - zb * d,  zb = sigmoid(-xs),  d = x1 - s, so the
linear algebra rides the (otherwise idle) PE:

  PE    pass 1: psum_xs = sum_k u_k*T_k  (diagonal stationaries u_k*I,
        T_k = window terms Ee,Eo,Oe,Oo); pass 2: psum_d = I@x1 +
        sum_k (-0.25*I)@T_k  = x1 - mean.
  ACT   zb = sigmoid(-f * psum_xs) straight out of PSUM (f = mask scale);
        d: PSUM -> SBUF fp16 copy.
  DVE   max pool (2 tensor_max, e-major intermediate) + final
        o = x1 - gq.
  GPSIMD  gq = zb * d.

A 3-stage software pipeline keeps every engine's in-order queue from
stalling on the cross-engine chain.  Inputs are staged fp16 on the host
(halves HBM read traffic); output returns fp16 and is upcast on host.
"""

import numpy as np

import concourse.bacc as bacc
import concourse.mybir as mybir
import concourse.tile as tile
from concourse.bass_utils import run_bass_kernel_spmd

F32 = mybir.dt.float32
F16 = mybir.dt.float16

B, H, W, C = 16, 256, 256, 64
N_CORES = 8
BPC = B // N_CORES          # batches per core
HO = H // 2                 # 128 output rows = SBUF partitions
NQ = 8                      # w-slices (tiles) per row
WQ = 16                     # output w per tile
N = WQ * C                  # 1024 free elems per partition per output tile
FD = 4 * N                  # input tile free dim (r2 * w16 * e2 * c64)
MMCH = 512                  # matmul moving-free chunk (PSUM bank)

LAST_EXEC_NS = None
LAST_RESULTS = None

_PROGRAM_CACHE = {}


def _build_program(bpc, nq, wq, ch):
    from contextlib import ExitStack

    n = wq * ch
    fd = 4 * n
    nch = n // MMCH             # psum chunks per tile

    nc = bacc.Bacc(
        "TRN2",
        target_bir_lowering=False,
        debug=False,
        enable_asserts=True,
        num_devices=N_CORES,
    )

    x = nc.dram_tensor("x", [bpc, HO, nq, fd], F16, kind="ExternalInput")
    wmat = nc.dram_tensor("wmat", [128, 6 * 128], F16, kind="ExternalInput")
    scal = nc.dram_tensor("scal", [128, 8], F32, kind="ExternalInput")
    out = nc.dram_tensor("out", [bpc, HO, nq, n], F16, kind="ExternalOutput")
    x_ap = x.ap()
    out_ap = out.ap()

    with tile.TileContext(nc) as tc, ExitStack() as ctx:
        pio = ctx.enter_context(tc.tile_pool(name="io", bufs=4))
        pbig = ctx.enter_context(tc.tile_pool(name="big", bufs=3))
        psm = ctx.enter_context(tc.tile_pool(name="small", bufs=4))
        pout = ctx.enter_context(tc.tile_pool(name="outp", bufs=3))
        pconst = ctx.enter_context(tc.tile_pool(name="const", bufs=1))
        ppsum = ctx.enter_context(tc.tile_pool(name="acc", bufs=2, space="PSUM"))

        Wt = pconst.tile([128, 6 * 128], F16)
        nc.sync.dma_start(Wt[:], wmat.ap()[:])
        Sc = pconst.tile([128, 8], F32)
        nc.sync.dma_start(Sc[:], scal.ap()[:])
        fneg_ap = Sc[:, 0:1]        # -f
        zero_ap = Sc[:, 1:2]
        Wd = [Wt[:, k * 128 : (k + 1) * 128] for k in range(6)]
        W_U = Wd[0:4]               # u_k * I
        W_Q = Wd[4]                 # -0.25 * I
        W_I = Wd[5]                 # I

        load_engines = [nc.sync, nc.scalar]
        load_rr = [0]

        def emit_load(b, q):
            EO = pio.tile([128, fd], F16, tag="EO")
            eng = load_engines[load_rr[0] % len(load_engines)]
            load_rr[0] += 1
            eng.dma_start(EO[:], x_ap[b, :, q, :])
            return dict(b=b, q=q, EO=EO)

        def terms_of(EO):
            EOv = EO[:].rearrange("p (r w e c) -> p r w e c", r=2, e=2, c=ch)

            def term(k, c0, nw):
                r, e = divmod(k, 2)
                return EOv[:, r, c0 : c0 + nw, e, :]

            return EOv, term

        wpc = MMCH // ch            # moving w's per chunk

        def emit_stage1(h):
            """xs psum + sigmoid; max pool."""
            b, q, EO = h["b"], h["q"], h["EO"]
            EOv, term = terms_of(EO)

            pxs = ppsum.tile([128, n], F32, tag="pxs")
            for cH in range(nch):
                for k in range(4):
                    nc.tensor.matmul(
                        pxs[:, cH * MMCH : (cH + 1) * MMCH],
                        W_U[k],
                        term(k, cH * wpc, wpc),
                        start=(k == 0),
                        stop=(k == 3),
                    )
            zb = psm.tile([128, n], F16, tag="zb")
            nc.scalar.activation(
                zb[:],
                pxs[:],
                mybir.ActivationFunctionType.Sigmoid,
                bias=zero_ap,
                scale=fneg_ap,
            )

            M1 = pbig.tile([128, 2 * n], F16, tag="M1")
            nc.vector.tensor_max(
                M1[:].rearrange("p (e w c) -> p w e c", e=2, c=ch),
                EOv[:, 0],
                EOv[:, 1],
            )
            x1 = psm.tile([128, n], F16, tag="x1")
            nc.vector.tensor_max(x1[:], M1[:, 0:n], M1[:, n : 2 * n])
            h.update(zb=zb, x1=x1)
            return h

        def emit_stage2(h):
            """d = x1 - mean: ACT seeds the psum with x1, PE accumulates
            the -0.25 window terms onto it."""
            EOv, term = terms_of(h["EO"])
            x1 = h["x1"]
            pd = ppsum.tile([128, n], F32, tag="pd")
            for cH in range(nch):
                nc.tensor.matmul(
                    pd[:, cH * MMCH : (cH + 1) * MMCH],
                    W_I,
                    x1[:, cH * MMCH : (cH + 1) * MMCH],
                    start=True,
                    stop=False,
                )
                for k in range(4):
                    nc.tensor.matmul(
                        pd[:, cH * MMCH : (cH + 1) * MMCH],
                        W_Q,
                        term(k, cH * wpc, wpc),
                        start=False,
                        stop=(k == 3),
                    )
            d = psm.tile([128, n], F16, tag="d")
            nc.scalar.copy(d[:], pd[:])
            h.update(d=d)
            return h

        def emit_stage3(h):
            """gq = zb*d (GPSIMD); o = x1 - gq; store."""
            gq = psm.tile([128, n], F16, tag="gq")
            nc.gpsimd.tensor_mul(gq[:], h["zb"][:], h["d"][:])
            o = pout.tile([128, n], F16, tag="o")
            nc.vector.tensor_sub(o[:], h["x1"][:], gq[:])
            nc.sync.dma_start(out_ap[h["b"], :, h["q"], :], o[:])

        tiles = [(b, q) for b in range(bpc) for q in range(nq)]
        ntiles = len(tiles)
        AHEAD = 3
        loaded = [emit_load(*tiles[i]) for i in range(min(AHEAD, ntiles))]
        s1q, s2q = [], []
        for i in range(ntiles):
            if i + AHEAD < ntiles:
                loaded.append(emit_load(*tiles[i + AHEAD]))
            s1q.append(emit_stage1(loaded.pop(0)))
            if len(s1q) >= 2:
                s2q.append(emit_stage2(s1q.pop(0)))
            if len(s2q) >= 2:
                emit_stage3(s2q.pop(0))
        while s1q:
            s2q.append(emit_stage2(s1q.pop(0)))
        while s2q:
            emit_stage3(s2q.pop(0))

    nc.compile()
    return nc


def _get_program(key):
    if key not in _PROGRAM_CACHE:
        _PROGRAM_CACHE[key] = _build_program(*key)
    return _PROGRAM_CACHE[key]


def _mask_consts(mask):
    """wmat [128, 6*128] f16 diagonal stationaries (u0..u3, -0.25*I, I)
    and scal [128, 8] f32 (-f, 0)."""
    m = np.asarray(mask, np.float64).reshape(-1)  # m00 m01 m10 m11 = Ee Eo Oe Oo
    f = float(m[np.argmax(np.abs(m))])
    if f == 0.0:
        f = 1.0
    u = m / f
    wmat = np.zeros((128, 6 * 128), np.float16)
    idx = np.arange(128)
    for k in range(4):
        wmat[idx, k * 128 + idx] = np.float16(u[k])
    wmat[idx, 4 * 128 + idx] = np.float16(-0.25)
    wmat[idx, 5 * 128 + idx] = np.float16(1.0)
    scal = np.zeros((128, 8), np.float32)
    scal[:, 0] = -f
    return wmat, scal


def kernel(x, mask):
    import os

    global LAST_EXEC_NS, LAST_RESULTS

    x = np.asarray(x)
    mask = np.asarray(mask)
    assert x.shape == (B, H, W, C), x.shape
    in_dtype = x.dtype

    wmat, scal = _mask_consts(mask)
    nc = _get_program((BPC, NQ, WQ, C))

    # stage as [b, h, q, r, w, e, c] -> fp16
    xs = np.asarray(x, np.float32).reshape(B, HO, 2, NQ, WQ, 2, C)
    xt = xs.transpose(0, 1, 3, 2, 4, 5, 6)
    xv = np.ascontiguousarray(xt).astype(np.float16).reshape(B, HO, NQ, FD)

    in_maps = [
        {"x": xv[i * BPC : (i + 1) * BPC], "wmat": wmat, "scal": scal}
        for i in range(N_CORES)
    ]

    trace = os.environ.get("KERNEL_TRACE", "0") == "1"
    res = run_bass_kernel_spmd(
        nc, in_maps, core_ids=list(range(N_CORES)), trace=trace
    )
    LAST_EXEC_NS = res.exec_time_ns
    LAST_RESULTS = res

    parts = [
        r["out"].reshape(BPC, HO, NQ * WQ, C).astype(np.float32)
        for r in res.results
    ]
    full = np.concatenate(parts, axis=0)
    return full.astype(in_dtype, copy=False)


def _numpy_reference(x, mask):
    xr = x.reshape(x.shape[0], x.shape[1] // 2, 2, x.shape[2] // 2, 2, x.shape[3])
    x1 = xr.max(axis=(2, 4))
    x2 = xr.mean(axis=(2, 4))
    xs = np.einsum("bhiwjc,ij->bhwc", xr, mask)
    z = 1.0 / (1.0 + np.exp(-xs))
    return z * x1 + (1.0 - z) * x2


if __name__ == "__main__":
    # Small-scale CoreSim self-test (no hardware needed).
    from concourse.bass_interp import CoreSim

    rng = np.random.default_rng(0)
    bpc_s, nq_s = 1, 2
    w_s = nq_s * WQ * 2
    xs_np = rng.standard_normal((bpc_s, H, w_s, C)).astype(np.float32)
    mask_np = (rng.standard_normal((2, 2)) * 0.5).astype(np.float32)

    wmat_s, scal_s = _mask_consts(mask_np)
    nc = _build_program(bpc_s, nq_s, WQ, C)
    sim = CoreSim(nc, trace=False)
    xr = xs_np.reshape(bpc_s, HO, 2, nq_s, WQ, 2, C).transpose(0, 1, 3, 2, 4, 5, 6)
    sim.tensor("x")[:] = (
        np.ascontiguousarray(xr).astype(np.float16).reshape(bpc_s, HO, nq_s, FD)
    )
    sim.tensor("wmat")[:] = wmat_s
    sim.tensor("scal")[:] = scal_s
    sim.simulate()
    got = sim.tensor("out").reshape(bpc_s, HO, nq_s * WQ, C).astype(np.float64)
    want = _numpy_reference(xs_np.astype(np.float64), mask_np.astype(np.float64))
    err = np.abs(got - want)
    rel = err.max() / np.abs(want).max()
    print("CoreSim selftest: max abs err", err.max(), "rel", rel)
    assert rel < 5e-3, rel
    print("PASS")
